# revision 1
# baseline (speedup 1.0000x reference)
"""Trainium2 Bass kernel for nn_Net_67765993996461.

Spiking CNN: conv2d -> LIF -> conv2d(dilated) -> LIF -> conv2d(dilated)
-> LIF -> time-mean -> FC.  Pure data parallel over batch: 32 images,
8 cores, 4 images/core.  Everything stays resident in SBUF per core.

Layout notes (per core, BL=4 local images):
- "scan space": partitions p = c + 64*(b%2), free = t*80 + (b//2)*40 + m.
- convs run as K-packed matmuls: two taps stacked along K=128 (partition
  blocks g=0/1 hold t-shifted copies of the spike map), and the two
  batch-parity groups run as col-tiled concurrent matmuls (tile_position
  (0,0)/(0,64)) writing psum partitions 0:64 / 64:128.
- LIF scan: per time step, 3 DVE ops on [128, 80] tiles:
    v = (v * a) + c        (scalar_tensor_tensor)
    s = (v >= 1)           (tensor_scalar is_ge, written to spike buffer)
    v = 0 where s          (copy_predicated)
- FC folds the time-mean: y = (wf @ sum_t s3) / 129 + bf.

Environment workarounds (this axon/fake_nrt runtime):
- walrus rejects multi-wait InstDrain -> split waits onto NOPs.
- branches hang -> merge all basic blocks into one (static code only).
- SP-engine DMAs with waits hang -> all DMAs issued from ACT (scalar).
"""
import sys

sys.path.insert(0, "/opt/trn_rl_repo")

import numpy as np
import ml_dtypes

import concourse.bass as bass
import concourse.mybir as mybir
from concourse import tile
from concourse.ap import AP
from concourse.bass_utils import run_bass_kernel_spmd

F32 = mybir.dt.float32
BF16 = mybir.dt.bfloat16
OP = mybir.AluOpType
AF = mybir.ActivationFunctionType

# ---------------- problem constants (hardcoded) ----------------
B, T0, M, C = 32, 128, 40, 64
NCORES = 8
BL = B // NCORES            # 4 images per core
T = T0 + 1                  # 129: conv1 output time length
TAU = np.float64(10.0) / 7.0
INV_TAU = float(1.0 / np.float32(TAU))
A_DECAY = float(np.float32(1.0) - np.float32(INV_TAU))   # 0.3

FS = 2 * M                  # 80 free elements per t-row in scan space
SL = T * FS                 # 10320

# conv2: rhs_dilation (4,3), padding (6,3): dt = 4i-6, dm = 3j-3
# conv3: rhs_dilation (16,9), padding (24,9): dt = 16i-24, dm = 9j-9
CONV2 = dict(dt0=(-6, 2), delta=4, dms=(-3, 0, 3), pt=6, pm=3)
CONV3 = dict(dt0=(-24, 8), delta=16, dms=(-9, 0, 9), pt=24, pm=9)
T1D, M1D = T + 12, M + 6     # 141, 46   s1 dup buffer dims
T2D, M2D = T + 48, M + 18    # 177, 58   s2 dup buffer dims
RFREE = max(BL * T1D * M1D, BL * T2D * M2D, 12 * 0 + BL * T * M)  # 41064
I1B = BL * T * M             # 20640 im2col cols

TCH = 6                      # conv chunk: t-rows per psum chunk
NCH = (T + TCH - 1) // TCH   # 22 chunks (last has 3 rows)

# ---------------- runtime-environment patches ----------------
from concourse.tile import ScopedClock
import concourse.tile as _tile


def _patched_drain_and_barrier(self, tick_clock, wait_clock):
    carrier = self.nc.sync.nop(nofuse=True, hint="tail_drain_waits")
    wait_clock.add_sem_waits(
        carrier.ins, ScopedClock({None: tick_clock.global_clock})
    )
    waits = list(carrier.ins.sync_info.on_wait) if carrier.ins.sync_info else []
    if len(waits) > 1:
        carrier.ins.sync_info = mybir.SyncInfo(on_wait=[waits[0]], on_update=[])
        for w in waits[1:]:
            extra = self.nc.sync.nop(nofuse=True, hint="tail_drain_waits")
            extra.ins.sync_info = mybir.SyncInfo(on_wait=[w], on_update=[])
    self.nc.sync.drain()
    self.nc.all_engine_barrier()
    assert self.sems is not None
    popped = self.nc._tile_sem_poison_stack.pop()
    assert popped is self._sem_poison
    self.nc.clear_and_free_semaphores(list(self.sems.allocated().values()))
    self.nc.all_engine_barrier()


_tile.TileContext._drain_and_barrier = _patched_drain_and_barrier


def merge_bbs(nc):
    """Post-process for this runtime: (a) flatten the linear bb chain into
    one bb (branches hang), dropping UnconditionalBranch; (b) split
    instructions carrying more than one sem-wait — this walrus build
    rejects multi-wait sync setup — by hoisting extra waits onto NoOps
    emitted just before on the same engine."""
    import json

    wseq = [0]

    def split_waits(ins, out_list):
        si = ins.get("sync_info")
        waits = (si or {}).get("on_wait") or []
        if len(waits) > 1:
            for w in waits[:-1]:
                wseq[0] += 1
                out_list.append({
                    "debug": ins.get("debug", 0), "engine": ins["engine"],
                    "ins": [], "name": f"WN-{wseq[0]}", "opcode": "NoOp",
                    "outs": [],
                    "sync_info": {"on_update": [], "on_wait": [w]},
                })
            si["on_wait"] = [waits[-1]]
        out_list.append(ins)

    j = json.loads(mybir.module_to_json_string(nc.m))
    for fn in j["functions"]:
        blocks = fn["blocks"]
        merged = []
        for bi, blk in enumerate(blocks):
            nxt = blocks[bi + 1]["name"] if bi + 1 < len(blocks) else None
            for ins in blk["instructions"]:
                if ins.get("opcode") == "UnconditionalBranch":
                    assert nxt is not None and ins["target"] == nxt
                    continue
                split_waits(ins, merged)
        blocks[0]["instructions"] = merged
        fn["blocks"] = [blocks[0]]
    nc.m = mybir.module_from_json_string(json.dumps(j))
    return nc


# ---------------- device kernel ----------------
def build_nc(debug=False, reps=1):
    nc = bass.Bass("TRN2", target_bir_lowering=False, debug=False)

    x_d = nc.declare_dram_parameter("x", [BL, 1, T0, M], F32, isOutput=False)
    w1_d = nc.declare_dram_parameter("w1p", [12, 128], F32, isOutput=False)
    w2_d = nc.declare_dram_parameter("w2p", [6, 128, 128], F32, isOutput=False)
    w3_d = nc.declare_dram_parameter("w3p", [6, 128, 128], F32, isOutput=False)
    fc_d = nc.declare_dram_parameter("fcp", [80, 32 * 12], F32, isOutput=False)
    bf_d = nc.declare_dram_parameter("bf", [12], F32, isOutput=False)
    y_d = nc.declare_dram_parameter("y", [BL, 12], F32, isOutput=True)
    if debug:
        dbg = {
            nm: nc.declare_dram_parameter(nm, [128, SL], F32, isOutput=True)
            for nm in ("s1o", "s2o", "s3o", "c1o", "c2o", "c3o")
        }

    with tile.TileContext(nc) as tc:
        with (
            tc.tile_pool(name="pool", bufs=1) as pool,
            tc.tile_pool(name="ppsum", bufs=2, space="PSUM") as ppsum,
            tc.tile_pool(name="pfc", bufs=1, space="PSUM") as pfc,
            tc.tile_pool(name="pdram", bufs=1, space="DRAM") as pdram,
        ):
            # ---- tiles ----
            xa = pool.tile([128, BL * M], F32)          # x as [t, (b, m)]
            xb = pool.tile([128, BL * M], BF16)
            R = pool.tile([128, RFREE], BF16)           # shared: I1 / s1d / s2d
            ct = pool.tile([128, SL], BF16)             # conv out -> scan input
            sp = pool.tile([128, SL], BF16)             # spike map (scan space)
            v = pool.tile([128, FS], F32)
            zero = pool.tile([128, FS], F32)
            wtmp = pool.tile([128, 768], F32)           # cast staging
            w1t = pool.tile([12, 128], BF16)
            w2t = pool.tile([128, 6 * 128], BF16)
            w3t = pool.tile([128, 6 * 128], BF16)
            fct = pool.tile([80, 32 * 12], BF16)
            bft = pool.tile([12, 1], F32)
            sbar = pool.tile([128, FS], F32)
            sbarb = pool.tile([128, FS], BF16)
            fcr = pool.tile([80, 32 * BL], BF16)
            ysb = pool.tile([12, BL], F32)
            scr = pdram.tile([BL, C * M], BF16)
            if debug:
                spf = pool.tile([128, SL], F32)

            DMA = nc.scalar.dma_start

            def rap(part0, nparts, offset, dims):
                """AP over R: partition range + free dims [[step, count]...]"""
                base = R[:]
                return AP(base.tensor, base.offset + part0 * RFREE + offset,
                          [[RFREE, nparts]] + [list(d) for d in dims])

            # ---- load x, weights ----
            DMA(AP(xa[:].tensor, xa[:].offset, [[BL * M, T0], [M, BL], [1, M]]),
                AP(x_d.ap().tensor, x_d.ap().offset,
                   [[M, T0], [T0 * M, BL], [1, M]]))   # dest [t | (b, m)]
            nc.vector.tensor_copy(xb[:], xa[:])
            DMA(wtmp[0:12, 0:128], w1_d[:])
            nc.vector.tensor_copy(w1t[:], wtmp[0:12, 0:128])
            DMA(wtmp[:, :],
                AP(w2_d.ap().tensor, 0, [[128, 128], [128 * 128, 6], [1, 128]]))
            nc.vector.tensor_copy(w2t[:], wtmp[:])
            DMA(wtmp[:, :],
                AP(w3_d.ap().tensor, 0, [[128, 128], [128 * 128, 6], [1, 128]]))
            nc.vector.tensor_copy(w3t[:], wtmp[:])
            DMA(wtmp[0:80, 0:384], fc_d[:])
            nc.vector.tensor_copy(fct[:], wtmp[0:80, 0:384])
            DMA(bft[:], AP(bf_d.ap().tensor, 0, [[1, 12], [1, 1]]))
            nc.gpsimd.memset(v[:], 0.0)
            nc.gpsimd.memset(zero[:], 0.0)

            # ---- build im2col I1 in R: I1[tap, b*T*M + t*M + m] ----
            nc.vector.memset(R[0:12, 0:I1B], 0.0)
            for i in range(4):
                for jj in range(3):
                    tap = i * 3 + jj
                    # out t range where t' = t+i-2 in [0, T0)
                    tlo = max(0, 2 - i)
                    thi = min(T, T0 + 2 - i)
                    mlo = max(0, 1 - jj)
                    mhi = min(M, M + 1 - jj)
                    for b in range(BL):
                        src = AP(xb[:].tensor,
                                 xb[:].offset + (tlo + i - 2) * (BL * M)
                                 + b * M + (mlo + jj - 1),
                                 [[BL * M, thi - tlo], [1, mhi - mlo]])
                        dst = rap(tap, 1, b * T * M + tlo * M + mlo,
                                  [[M, thi - tlo], [1, mhi - mlo]])
                        DMA(dst, src)

            # ---- conv layer runner ----
            def conv_chunks(lhsT_tile, K, rhs_ap_fn, nmm):
                """for each t-chunk: accumulate nmm tap-matmuls x 2 halves
                into psum, then ACT-drain into ct (scale already in W)."""
                for ch in range(NCH):
                    u0 = ch * TCH
                    tc_ = min(TCH, T - u0)
                    ncols = tc_ * FS // 2 * 2  # = tc_*80; per half tc_*80... cols per half:
                    nhalf = 2 * tc_ * M        # (b2, t, m) = 2*tc_*40
                    pc = ppsum.tile([128, TCH * FS // 2 * 2], F32, tag="pc")
                    for mm in range(nmm):
                        for half in range(2):
                            rhs = rhs_ap_fn(mm, half, u0, tc_)
                            nc.tensor.matmul(
                                pc[half * 64:(half + 1) * 64, 0:nhalf],
                                lhsT_tile[0:K, mm * 128 + half * 64:
                                          mm * 128 + half * 64 + 64],
                                rhs,
                                start=(mm == 0), stop=(mm == nmm - 1),
                                tile_position=(0, half * 64))
                    # drain psum -> ct rows u0..u0+tc_
                    src = AP(pc[:].tensor, pc[:].offset,
                             [[TCH * FS, 128], [tc_ * M, 2], [M, tc_], [1, M]])
                    dst = AP(ct[:].tensor, ct[:].offset + u0 * FS,
                             [[SL, 128], [M, 2], [FS, tc_], [1, M]])
                    nc.scalar.activation(dst, src, AF.Copy, scale=1.0)

            def conv1_rhs(mm, half, u0, tc_):
                return rap(0, 12, u0 * M + half * T * M,
                           [[2 * T * M, 2], [M, tc_], [1, M]])

            def mk_conv_rhs(geom, TD, MD):
                BS = TD * MD
                def fn(mm, half, u0, tc_):
                    tp, jj = divmod(mm, 3)
                    off = (half * BS
                           + (geom["pt"] + geom["dt0"][tp] + u0) * MD
                           + (geom["pm"] + geom["dms"][jj]))
                    return AP(R[:].tensor, R[:].offset + off,
                              [[RFREE, 128], [2 * BS, 2], [MD, tc_], [1, M]])
                return fn

            # ---- LIF scan ----
            def lif_scan(layer):
                for t in range(T):
                    csl = ct[:, t * FS:(t + 1) * FS]
                    ssl = sp[:, t * FS:(t + 1) * FS]
                    nc.vector.scalar_tensor_tensor(
                        out=v[:], in0=v[:], scalar=A_DECAY, in1=csl,
                        op0=OP.mult, op1=OP.add)
                    nc.vector.tensor_scalar(
                        out=ssl, in0=v[:], scalar1=1.0, scalar2=None,
                        op0=OP.is_ge)
                    nc.vector.copy_predicated(
                        out=v[:], mask=ssl.bitcast(mybir.dt.uint16),
                        data=zero[:])

            def dump(name_s, name_c):
                if not debug:
                    return
                nc.vector.tensor_copy(spf[:], sp[:])
                DMA(dbg[name_s].ap(), spf[:])
                nc.vector.tensor_copy(spf[:], ct[:])
                DMA(dbg[name_c].ap(), spf[:])

            # ---- dup DMAs: sp -> R as s?d (K-pack layout) ----
            def dup_spikes(pt_, delta, TD, MD):
                BS = TD * MD
                # zero pad rows [0, pt_) and [T, TD) and m-strips
                pm_ = (MD - M) // 2
                nc.vector.memset(
                    AP(R[:].tensor, R[:].offset,
                       [[RFREE, 128], [BS, BL], [1, pt_ * MD]]), 0.0)
                nc.vector.memset(
                    AP(R[:].tensor, R[:].offset + T * MD,
                       [[RFREE, 128], [BS, BL], [1, (TD - T) * MD]]), 0.0)
                nc.vector.memset(
                    AP(R[:].tensor, R[:].offset,
                       [[RFREE, 128], [BS, BL], [MD, TD], [1, pm_]]), 0.0)
                nc.vector.memset(
                    AP(R[:].tensor, R[:].offset + pm_ + M,
                       [[RFREE, 128], [BS, BL], [MD, TD], [1, pm_]]), 0.0)
                for bh in range(2):
                    for g in range(2):
                        for b2 in range(2):
                            src = AP(sp[:].tensor,
                                     sp[:].offset + bh * 64 * SL + b2 * M,
                                     [[SL, 64], [FS, T], [1, M]])
                            dst = AP(R[:].tensor,
                                     R[:].offset + g * 64 * RFREE
                                     + (2 * b2 + bh) * BS
                                     + (pt_ - g * delta) * MD + pm_,
                                     [[RFREE, 64], [MD, T], [1, M]])
                            DMA(dst, src)

            # ================= emission =================
            for _rep in range(reps):
                nc.gpsimd.memset(v[:], 0.0)
                conv_chunks(w1t, 12, conv1_rhs, 1)
                lif_scan(1)
                dump("s1o", "c1o")
                dup_spikes(CONV2["pt"], CONV2["delta"], T1D, M1D)
                conv_chunks(w2t, 128, mk_conv_rhs(CONV2, T1D, M1D), 6)
                lif_scan(2)
                dump("s2o", "c2o")
                dup_spikes(CONV3["pt"], CONV3["delta"], T2D, M2D)
                conv_chunks(w3t, 128, mk_conv_rhs(CONV3, T2D, M2D), 6)
                lif_scan(3)
                dump("s3o", "c3o")

            # ---- time-sum of s3 -> sbar [128, 80] ----
            nc.vector.tensor_reduce(
                sbar[:],
                AP(sp[:].tensor, sp[:].offset, [[SL, 128], [M, 2], [1, M], [FS, T]]),
                axis=mybir.AxisListType.X, op=OP.add)

            # ---- FC: y = (wf @ sbar)/T + bf ----
            nc.vector.tensor_copy(sbarb[:], sbar[:])
            for bh in range(2):
                src = AP(sbarb[:].tensor, sbarb[:].offset + bh * 64 * FS,
                         [[FS, 64], [M, 2], [1, M]])
                dst = AP(scr[:].tensor, scr[:].offset + bh * C * M,
                         [[M, 64], [2 * C * M, 2], [1, M]])
                DMA(dst, src)
            with nc.allow_non_contiguous_dma(reason="tiny fc relayout"):
                for b in range(BL):
                    DMA(AP(fcr[:].tensor, fcr[:].offset + b * 32,
                           [[32 * BL, 80], [1, 32]]),
                        AP(scr[:].tensor, scr[:].offset + b * C * M,
                           [[1, 80], [80, 32]]))
            pf = pfc.tile([12, BL], F32)
            for k in range(32):
                nc.tensor.matmul(
                    pf[:, :], fct[0:80, k * 12:(k + 1) * 12],
                    AP(fcr[:].tensor, fcr[:].offset + k, [[32 * BL, 80], [32, BL]]),
                    start=(k == 0), stop=(k == 31))
            nc.scalar.activation(ysb[:], pf[:, :], AF.Identity,
                                 bias=bft[:, 0:1], scale=float(1.0 / 129.0))
            DMA(AP(y_d.ap().tensor, 0, [[1, 12], [12, BL]]), ysb[:])

    return nc


# ---------------- host-side weight packing ----------------
def pack_inputs(x, w1, w2, w3, wf, bf):
    """Returns list of per-core input maps."""
    inv_tau = np.float32(INV_TAU)
    w1p = np.zeros((12, 128), np.float32)
    for i in range(4):
        for jj in range(3):
            w1p[i * 3 + jj, 0:64] = w1[:, 0, i, jj] * inv_tau
    w1p[:, 64:128] = w1p[:, 0:64]

    def pack_w(w):
        wp = np.zeros((6, 128, 128), np.float32)
        for tp in range(2):
            for jj in range(3):
                mm = tp * 3 + jj
                for g in range(2):
                    i = tp * 2 + g
                    blk = w[:, :, i, jj].T * inv_tau   # [c_in, c_out]
                    wp[mm, g * 64:(g + 1) * 64, 0:64] = blk
                    wp[mm, g * 64:(g + 1) * 64, 64:128] = blk
        return wp

    w2p = pack_w(w2)
    w3p = pack_w(w3)
    fcp = np.zeros((80, 32 * 12), np.float32)
    for k in range(32):
        fcp[:, k * 12:(k + 1) * 12] = wf[:, 80 * k:80 * (k + 1)].T
    bfv = np.asarray(bf, np.float32).reshape(12)

    maps = []
    for c in range(NCORES):
        maps.append({
            "x": np.ascontiguousarray(x[c * BL:(c + 1) * BL], np.float32),
            "w1p": w1p, "w2p": w2p, "w3p": w3p, "fcp": fcp, "bf": bfv,
        })
    return maps


_CACHED = {}


def get_nc(debug=False, reps=1):
    key = (bool(debug), reps)
    if key not in _CACHED:
        nc = build_nc(debug=debug, reps=reps)
        merge_bbs(nc)
        _CACHED[key] = nc
    return _CACHED[key]


def make_runner(nc, in_maps):
    """Build the sharded PJRT callable once (mimics bass2jax.run_bass_via_pjrt)
    so repeated calls reuse the compiled executable for timing."""
    import jax
    from jax.sharding import Mesh, PartitionSpec
    from jax.experimental.shard_map import shard_map
    from concourse import bass2jax
    from concourse.bass2jax import _bass_exec_p, install_neuronx_cc_hook, partition_id_tensor

    install_neuronx_cc_hook()
    n_cores = len(in_maps)
    partition_name = nc.partition_id_tensor.name if nc.partition_id_tensor else None
    in_names, out_names, out_avals, zero_outs = [], [], [], []
    for alloc in nc.m.functions[0].allocations:
        if not isinstance(alloc, mybir.MemoryLocationSet):
            continue
        name = alloc.memorylocations[0].name
        if alloc.kind == "ExternalInput":
            if name != partition_name:
                in_names.append(name)
        elif alloc.kind == "ExternalOutput":
            out_names.append(name)
            shape = tuple(alloc.tensor_shape)
            dtype = mybir.dt.np(alloc.dtype)
            out_avals.append(jax.core.ShapedArray(shape, dtype))
            zero_outs.append(np.zeros(shape, dtype))
    n_params = len(in_names)
    n_outs = len(out_avals)
    in_names_all = in_names + out_names + ([partition_name] if partition_name else [])

    def _body(*args):
        operands = list(args)
        if partition_name is not None:
            operands.append(partition_id_tensor())
        outs = _bass_exec_p.bind(
            *operands,
            out_avals=tuple(out_avals),
            in_names=tuple(in_names_all),
            out_names=tuple(out_names),
            lowering_input_output_aliases=(),
            sim_require_finite=True,
            sim_require_nnan=True,
            nc=nc,
        )
        return tuple(outs)

    devices = jax.devices()[:n_cores]
    mesh = Mesh(np.asarray(devices), ("core",))
    donate = tuple(range(n_params, n_params + n_outs))
    sharded = jax.jit(
        shard_map(_body, mesh=mesh,
                  in_specs=(PartitionSpec("core"),) * (n_params + n_outs),
                  out_specs=(PartitionSpec("core"),) * n_outs,
                  check_rep=False),
        donate_argnums=donate, keep_unused=True)
    concat_in = [
        np.concatenate([np.asarray(in_maps[c][nm]) for c in range(n_cores)], axis=0)
        for nm in in_names
    ]

    def run():
        zeros = [np.zeros((n_cores * z.shape[0], *z.shape[1:]), z.dtype)
                 for z in zero_outs]
        out_arrs = sharded(*concat_in, *zeros)
        out_arrs = [np.asarray(a) for a in out_arrs]
        return [
            {nm: out_arrs[i].reshape(n_cores, *out_avals[i].shape)[c]
             for i, nm in enumerate(out_names)}
            for c in range(n_cores)
        ]

    return run


def kernel(x, w1, w2, w3, wf, bf):
    nc = get_nc(debug=False)
    in_maps = pack_inputs(np.asarray(x), np.asarray(w1), np.asarray(w2),
                          np.asarray(w3), np.asarray(wf), np.asarray(bf))
    res = run_bass_kernel_spmd(nc, in_maps, list(range(NCORES)))
    y = np.concatenate([res.results[c]["y"] for c in range(NCORES)], axis=0)
    return y.astype(np.float32)

